# revision 16
# baseline (speedup 1.0000x reference)
"""MoE (top-2 of 8 experts, SwiGLU MLP) on 8 Trainium2 NeuronCores.

Strategy: expert-parallel with host-side routing.  The host computes the
gate (f64 scores -> top-2 -> softmax) and quantizes x / w1 / w3 / w2 to
fp8e4m3 hi/lo pairs at power-of-2 scales.  On each core every GEMM runs as
fp8e4 DoubleRow matmuls (two contraction tiles per pass, 0.5 cycles/row)
with the 3-term hi/lo expansion  W*x ~= Wh*xh + Wl*xh + Wh*xl, i.e. 0.75x
the fp32r cycle count; rel err ~5e-3 vs the 2e-2 gate.  silu runs on the
scalar engine with the PSUM descale folded into its input scale; act is
split hi/lo with a DVE mul, a scalar-engine fp8 cast and a mixed-dtype DVE
subtract.  The host scatter-adds the weighted per-expert outputs back,
folding the fp8 output scale into the combine weights.  Warm-up matmuls
cover the PE p-state ramp during the initial DMA fill; x streams in k-pair
granularity consumed k-pair-major by the first three f-tiles (the third
borrowing the idle psy PSUM banks); the smaller slot runs first so the
program tail ends on a tiny chunk.

Load balance uses an asymmetric
two-slot load balancing: every core runs two sequential MLP "slots" of
capacities (CH1, CH2); every expert's token set is split into two pieces
placed in slots on two different cores.  This cuts the per-core token
capacity from max_e C_e (1072) to roughly mean+pad (CH1+CH2 = 1040 for the
seed-0 routing), at the cost of streaming two experts' weights per core.

Slot solver: CH1 = pad(max_e ceil(C_e/2)); find the smallest CH2 such that
with n11 = n22 experts on (slot1,slot1) / (slot2,slot2) core pairs and the
rest on (slot1,slot2), all pieces fit.  Every feasible config satisfies
CH1+CH2 >= max_e C_e, so this is optimal for <=2 slots per core.
"""

import math

import ml_dtypes
import numpy as np

import concourse.bass as bass  # noqa: F401  (registers AP machinery)
import concourse.tile as tile
from concourse import bacc, mybir
from concourse.bass_utils import run_bass_kernel_spmd

P = 128
H = 1024
F = 4096
E = 8
TOPK = 2
N_CORES = 8

KO = H // P
FO = F // P
HO = H // P
FG = 16
NG = FO // FG

F32 = mybir.dt.float32
F8 = mybir.dt.float8e4
FP8 = ml_dtypes.float8_e4m3
DR = mybir.MatmulPerfMode.DoubleRow

SX = 4.0
SW1 = 128.0
SW3 = 8.0
SW2 = 16.0
SA = SW3 * SX
SILU_SCALE = 1.0 / (SW1 * SX)
YSCALE = 1.0 / (SA * SW2)

W13_STRIDE = FO * 4 * KO * P
W2_STRIDE = HO * 2 * FO * P

_NC_CACHE: dict = {}


def _chunks(C: int):
    out, off = [], 0
    while off < C:
        cw = min(512, C - off)
        out.append((off, cw))
        off += cw
    return out


def _pad8(n):
    return max(8, math.ceil(n / 8) * 8)


def _solve_slots(counts):
    """Return (CH1, CH2, assign) where assign[core] = [(e, lo, hi), (e, lo, hi)]
    giving the token sub-range of expert e in each slot."""
    order = np.argsort(-np.asarray(counts), kind="stable")
    CH1 = _pad8(max(math.ceil(c / 2) for c in counts))
    lo = _pad8(max(8, math.ceil(sum(counts) / N_CORES) - CH1))
    for CH2 in range(lo, CH1 + 8, 8):
        k = sum(1 for c in counts if c > CH1 + CH2)
        for n11 in range(k, 5):
            n22 = n11
            n12 = 8 - 2 * n11
            if n12 < 0:
                continue
            sorted_c = [counts[e] for e in order]
            ok = all(c <= 2 * CH1 for c in sorted_c[:n11])
            ok = ok and all(c <= CH1 + CH2
                            for c in sorted_c[n11:n11 + n12])
            ok = ok and all(c <= 2 * CH2 for c in sorted_c[n11 + n12:])
            if not ok:
                continue
            # build assignment
            assign = [[None, None] for _ in range(N_CORES)]
            s1_free = list(range(N_CORES))
            s2_free = list(range(N_CORES))
            idx = 0
            for i in range(n11):
                e = order[idx]; idx += 1
                c1, c2 = s1_free.pop(0), s1_free.pop(0)
                a = min(counts[e], CH1)
                assign[c1][0] = (e, 0, a)
                assign[c2][0] = (e, a, counts[e])
            for i in range(n12):
                e = order[idx]; idx += 1
                c1 = s1_free.pop(0)
                c2 = next(c for c in s2_free if c != c1)
                s2_free.remove(c2)
                a = min(counts[e], CH1)
                assign[c1][0] = (e, 0, a)
                assign[c2][1] = (e, a, counts[e])
            for i in range(n22):
                e = order[idx]; idx += 1
                c1, c2 = s2_free.pop(0), s2_free.pop(0)
                a = min(counts[e], CH2)
                assign[c1][1] = (e, 0, a)
                assign[c2][1] = (e, a, counts[e])
            for c in range(N_CORES):
                if assign[c][0] is None:
                    assign[c][0] = (0, 0, 0)
                if assign[c][1] is None:
                    assign[c][1] = (0, 0, 0)
            return CH1, CH2, assign
    raise RuntimeError("no slot config found")


def _build_nc(CH1: int, CH2: int):
    CT = CH1 + CH2
    caps = (CH1, CH2)

    nc = bacc.Bacc("TRN2", target_bir_lowering=False, debug=False,
                   num_devices=N_CORES)
    xh = nc.dram_tensor("xh", [H, CT], F8, kind="ExternalInput").ap()
    xl = nc.dram_tensor("xl", [H, CT], F8, kind="ExternalInput").ap()
    w13 = nc.dram_tensor("w13", [P, 2 * W13_STRIDE], F8,
                         kind="ExternalInput").ap()
    w2p = nc.dram_tensor("w2p", [P, 2 * W2_STRIDE], F8,
                         kind="ExternalInput").ap()
    yT = nc.dram_tensor("yT", [H, CT], F32, kind="ExternalOutput").ap()

    xh_t = xh.rearrange("(ko p) c -> p ko c", p=P)
    xl_t = xl.rearrange("(ko p) c -> p ko c", p=P)
    w13_t = w13.rearrange("p (s fo t ko q) -> p s fo t ko q",
                          s=2, fo=FO, t=4, ko=KO, q=P)
    w2_t = w2p.rearrange("p (s ho t fo q) -> p s ho t fo q",
                         s=2, ho=HO, t=2, fo=FO, q=P)
    yT_t = yT.rearrange("(ho p) c -> p ho c", p=P)

    with tile.TileContext(nc) as tc:
        with (
            tc.tile_pool(name="xres", bufs=1) as xpool,
            tc.tile_pool(name="yres", bufs=1) as ypool,
            tc.tile_pool(name="actres", bufs=1) as actpool,
            tc.tile_pool(name="w13", bufs=5) as w13pool,
            tc.tile_pool(name="w2p", bufs=3) as w2pool,
            tc.tile_pool(name="tmp", bufs=4) as tmppool,
            tc.tile_pool(name="psh", bufs=2, space="PSUM") as ps_h,
            tc.tile_pool(name="psu", bufs=2, space="PSUM") as ps_u,
            tc.tile_pool(name="psy", bufs=4, space="PSUM") as ps_y,
        ):
            w13_tiles = {}

            def load_w13(s, fo):
                w_f = w13pool.tile([P, 4, KO, P], F8, tag="w13",
                                   name=f"w13_s{s}f{fo}")
                nc.sync.dma_start(w_f[:], w13_t[:, s, fo])
                w13_tiles[(s, fo)] = w_f

            warm_sb = xpool.tile([P, 2, P], F8, tag="warm")
            nc.vector.memset(warm_sb[:], 0)

            def emit_warms(n):
                # p-state keep-alive: fills PE idle while DMA streams in,
                # so the 1.2GHz->2.4GHz ramp never resets
                for _ in range(n):
                    warm_ps = ps_y.tile([P, 512], F32, tag="psy",
                                        name="warm")
                    nc.tensor.matmul(warm_ps[:, :P], warm_sb[:], warm_sb[:],
                                     start=True, stop=True, perf_mode=DR)

            emit_warms(96)

            xh_sb = xpool.tile([P, KO, CT], F8, tag="xh")
            xl_sb = xpool.tile([P, KO, CT], F8, tag="xl")
            load_w13(1, 0)
            load_w13(1, 1)
            load_w13(1, 2)
            for kp in range(KO // 2):
                sl = slice(2 * kp, 2 * kp + 2)
                nc.sync.dma_start(xh_sb[:, sl], xh_t[:, sl])
                nc.sync.dma_start(xl_sb[:, sl], xl_t[:, sl])
            load_w13(1, 3)

            y_sb = ypool.tile([P, HO, CT], F32)
            acth_sb = actpool.tile([P, FG, CH1], F8, tag="acth")
            actl_sb = actpool.tile([P, FG, CH1], F8, tag="actl")

            def emit_terms(ps, wh, wl, kp, nkp, off, cw, xoff=0):
                sl = slice(2 * kp, 2 * kp + 2)
                first, last = kp == 0, kp == nkp - 1
                nc.tensor.matmul(ps[:, :cw], wh[:, sl],
                                 xh_sb[:, sl, xoff + off:xoff + off + cw],
                                 start=first, stop=False, perf_mode=DR)
                nc.tensor.matmul(ps[:, :cw], wl[:, sl],
                                 xh_sb[:, sl, xoff + off:xoff + off + cw],
                                 start=False, stop=False, perf_mode=DR)
                nc.tensor.matmul(ps[:, :cw], wh[:, sl],
                                 xl_sb[:, sl, xoff + off:xoff + off + cw],
                                 start=False, stop=last, perf_mode=DR)

            def emit_act(fi, h_ps, u_ps, off, cw):
                s_sb = tmppool.tile([P, 512], F32, tag="silu")
                nc.scalar.activation(s_sb[:, :cw], h_ps[:, :cw],
                                     mybir.ActivationFunctionType.Silu,
                                     scale=SILU_SCALE)
                a_sb = tmppool.tile([P, 512], F32, tag="actf")
                nc.vector.tensor_mul(a_sb[:, :cw], s_sb[:, :cw],
                                     u_ps[:, :cw])
                nc.scalar.activation(acth_sb[:, fi, off:off + cw],
                                     a_sb[:, :cw],
                                     mybir.ActivationFunctionType.Copy)
                nc.vector.tensor_sub(actl_sb[:, fi, off:off + cw],
                                     a_sb[:, :cw],
                                     acth_sb[:, fi, off:off + cw])

            def emit_f_pair_kp_major(w_fs, chunks, xoff):
                """First f-tiles of the first slot: k-pair-major across all
                of them per chunk so the PE consumes x k-pairs in DMA
                arrival order with enough work per pair to avoid stalls.
                The third tile (single-chunk slots only) borrows the psy
                banks, which the warm-up matmuls have drained by then."""
                pools = [(ps_h, "h"), (ps_h, "h"), (ps_y, "psy")]
                upools = [(ps_u, "u"), (ps_u, "u"), (ps_y, "psy")]
                for off, cw in chunks:
                    tiles = [(w_f, fi,
                              pools[j][0].tile([P, 512], F32, tag=pools[j][1],
                                               name=f"f0h_{fi}_{off}"),
                              upools[j][0].tile([P, 512], F32,
                                                tag=upools[j][1],
                                                name=f"f0u_{fi}_{off}"))
                             for j, (w_f, fi) in enumerate(w_fs)]
                    for kp in range(KO // 2):
                        for w_f, fi, h_ps, u_ps in tiles:
                            emit_terms(h_ps, w_f[:, 0], w_f[:, 1],
                                       kp, KO // 2, off, cw, xoff)
                            emit_terms(u_ps, w_f[:, 2], w_f[:, 3],
                                       kp, KO // 2, off, cw, xoff)
                    for w_f, fi, h_ps, u_ps in tiles:
                        emit_act(fi, h_ps, u_ps, off, cw)

            for s in (1, 0):
                col0 = 0 if s == 0 else CH1
                Cs = caps[s]
                chunks = _chunks(Cs)
                for g in range(NG):
                    f0 = g * FG
                    for fi in range(FG):
                        fo = f0 + fi
                        n_lead = 3 if len(chunks) == 1 else 2
                        if s == 1 and g == 0 and fi == 0:
                            emit_f_pair_kp_major(
                                [(w13_tiles.pop((1, j)), j)
                                 for j in range(n_lead)], chunks, col0)
                            continue
                        if s == 1 and g == 0 and fi < n_lead:
                            continue
                        if (s, fo) not in w13_tiles:
                            load_w13(s, fo)
                        w_f = w13_tiles.pop((s, fo))
                        for ci, (off, cw) in enumerate(chunks):
                            x0 = col0 + off
                            h_ps = ps_h.tile([P, 512], F32, tag="h")
                            u_ps = ps_u.tile([P, 512], F32, tag="u")
                            for kp in range(KO // 2):
                                sl = slice(2 * kp, 2 * kp + 2)
                                first, last = kp == 0, kp == KO // 2 - 1
                                nc.tensor.matmul(
                                    h_ps[:, :cw], w_f[:, 0, sl],
                                    xh_sb[:, sl, x0:x0 + cw],
                                    start=first, stop=False, perf_mode=DR)
                                nc.tensor.matmul(
                                    h_ps[:, :cw], w_f[:, 1, sl],
                                    xh_sb[:, sl, x0:x0 + cw],
                                    start=False, stop=False, perf_mode=DR)
                                nc.tensor.matmul(
                                    h_ps[:, :cw], w_f[:, 0, sl],
                                    xl_sb[:, sl, x0:x0 + cw],
                                    start=False, stop=last, perf_mode=DR)
                            for kp in range(KO // 2):
                                sl = slice(2 * kp, 2 * kp + 2)
                                first, last = kp == 0, kp == KO // 2 - 1
                                nc.tensor.matmul(
                                    u_ps[:, :cw], w_f[:, 2, sl],
                                    xh_sb[:, sl, x0:x0 + cw],
                                    start=first, stop=False, perf_mode=DR)
                                nc.tensor.matmul(
                                    u_ps[:, :cw], w_f[:, 3, sl],
                                    xh_sb[:, sl, x0:x0 + cw],
                                    start=False, stop=False, perf_mode=DR)
                                nc.tensor.matmul(
                                    u_ps[:, :cw], w_f[:, 2, sl],
                                    xl_sb[:, sl, x0:x0 + cw],
                                    start=False, stop=last, perf_mode=DR)
                            s_sb = tmppool.tile([P, 512], F32, tag="silu")
                            nc.scalar.activation(
                                s_sb[:, :cw], h_ps[:, :cw],
                                mybir.ActivationFunctionType.Silu,
                                scale=SILU_SCALE)
                            a_sb = tmppool.tile([P, 512], F32, tag="actf")
                            nc.vector.tensor_mul(
                                a_sb[:, :cw], s_sb[:, :cw], u_ps[:, :cw])
                            nc.scalar.activation(
                                acth_sb[:, fi, off:off + cw], a_sb[:, :cw],
                                mybir.ActivationFunctionType.Copy)
                            nc.vector.tensor_sub(
                                actl_sb[:, fi, off:off + cw], a_sb[:, :cw],
                                acth_sb[:, fi, off:off + cw])
                    if s == 1 and g == NG - 1:
                        # prefetch the second slot's first weight tiles ahead
                        # of this slot's down-phase DMA traffic
                        load_w13(0, 0)
                        load_w13(0, 1)
                        load_w13(0, 2)
                    for ho in range(HO):
                        w2_h = w2pool.tile([P, 2, FG, P], F8, tag="w2")
                        nc.sync.dma_start(w2_h[:], w2_t[:, s, ho, :,
                                                       f0:f0 + FG])
                        for off, cw in chunks:
                            x0 = col0 + off
                            y_ps = ps_y.tile([P, 512], F32, tag="psy")
                            for fp in range(FG // 2):
                                sl = slice(2 * fp, 2 * fp + 2)
                                first, last = fp == 0, fp == FG // 2 - 1
                                nc.tensor.matmul(
                                    y_ps[:, :cw], w2_h[:, 0, sl],
                                    acth_sb[:, sl, off:off + cw],
                                    start=first, stop=False, perf_mode=DR)
                                nc.tensor.matmul(
                                    y_ps[:, :cw], w2_h[:, 1, sl],
                                    acth_sb[:, sl, off:off + cw],
                                    start=False, stop=False, perf_mode=DR)
                                nc.tensor.matmul(
                                    y_ps[:, :cw], w2_h[:, 0, sl],
                                    actl_sb[:, sl, off:off + cw],
                                    start=False, stop=last, perf_mode=DR)
                            if g == 0:
                                nc.vector.tensor_copy(
                                    y_sb[:, ho, x0:x0 + cw], y_ps[:, :cw])
                            else:
                                nc.vector.tensor_add(
                                    y_sb[:, ho, x0:x0 + cw],
                                    y_sb[:, ho, x0:x0 + cw], y_ps[:, :cw])
                                last_tail = s == 0 and ho == HO - 1
                                if not last_tail:
                                    if (off, cw) == chunks[-1]:
                                        nc.sync.dma_start(
                                            yT_t[:, ho, col0:col0 + Cs],
                                            y_sb[:, ho, col0:col0 + Cs])
                                else:
                                    nc.sync.dma_start(
                                        yT_t[:, ho, x0:x0 + cw],
                                        y_sb[:, ho, x0:x0 + cw])

    nc.compile()
    return nc


def _route(x, gate_w):
    xt = x.reshape(-1, H)
    scores = xt.astype(np.float64) @ gate_w.astype(np.float64).T
    ei = np.argsort(-scores, axis=1, kind="stable")[:, :TOPK]
    ev = np.take_along_axis(scores, ei, axis=1)
    ev = ev - ev.max(axis=1, keepdims=True)
    ew = np.exp(ev)
    ew = ew / ew.sum(axis=1, keepdims=True)
    routes = []
    for e in range(E):
        mask = ei == e
        toks = np.nonzero(mask.any(axis=1))[0]
        wts = (ew * mask).sum(axis=1)[toks]
        routes.append((toks, wts.astype(np.float32)))
    return routes


def _qpair(v, S):
    vs = v * np.float32(S)
    hi = np.asarray(vs, dtype=FP8)
    lo = np.asarray(vs - hi.astype(np.float32), dtype=FP8)
    return hi, lo


def _pack_w13(w1, w3):
    w1h, w1l = _qpair(w1, SW1)
    w3h, w3l = _qpair(w3, SW3)
    planes = [a.reshape(KO, P, FO, P).transpose(1, 2, 0, 3)
              for a in (w1h, w1l, w3h, w3l)]
    packed = np.stack(planes, axis=2)          # [P, FO, 4, KO, P]
    return np.ascontiguousarray(packed).reshape(P, -1)


def _pack_w2(w2):
    w2h, w2l = _qpair(w2, SW2)
    planes = [a.reshape(FO, P, HO, P).transpose(1, 2, 0, 3)
              for a in (w2h, w2l)]             # [P, HO, FO, P]
    packed = np.stack(planes, axis=2)          # [P, HO, 2, FO, P]
    return np.ascontiguousarray(packed).reshape(P, -1)


def _run(inputs, trace=False, trace_kwargs=None):
    x = np.ascontiguousarray(np.asarray(inputs["x"], dtype=np.float32))
    gate_w = np.asarray(inputs["gate_w"], dtype=np.float32)
    w1 = np.asarray(inputs["w1"], dtype=np.float32)
    w3 = np.asarray(inputs["w3"], dtype=np.float32)
    w2 = np.asarray(inputs["w2"], dtype=np.float32)
    B, S, Hd = x.shape
    assert Hd == H and w1.shape == (E, H, F) and w2.shape == (E, F, H)

    routes = _route(x, gate_w)
    counts = [len(toks) for toks, _ in routes]
    CH1, CH2, assign = _solve_slots(counts)
    CT = CH1 + CH2

    if (CH1, CH2) not in _NC_CACHE:
        _NC_CACHE[(CH1, CH2)] = _build_nc(CH1, CH2)
    nc = _NC_CACHE[(CH1, CH2)]

    w13_packs = {}
    w2_packs = {}
    xt = x.reshape(-1, H)
    in_maps = []
    for c in range(N_CORES):
        xT_c = np.zeros((H, CT), dtype=np.float32)
        w13_cat = np.zeros((P, 2 * W13_STRIDE), dtype=FP8)
        w2_cat = np.zeros((P, 2 * W2_STRIDE), dtype=FP8)
        for s, col0 in ((0, 0), (1, CH1)):
            e, lo, hi = assign[c][s]
            toks = routes[e][0][lo:hi]
            xT_c[:, col0:col0 + len(toks)] = xt[toks].T
            if e not in w13_packs:
                w13_packs[e] = _pack_w13(w1[e], w3[e])
                w2_packs[e] = _pack_w2(w2[e])
            w13_cat[:, s * W13_STRIDE:(s + 1) * W13_STRIDE] = w13_packs[e]
            w2_cat[:, s * W2_STRIDE:(s + 1) * W2_STRIDE] = w2_packs[e]
        xh8, xl8 = _qpair(xT_c, SX)
        in_maps.append({"xh": xh8, "xl": xl8, "w13": w13_cat, "w2p": w2_cat})

    res = run_bass_kernel_spmd(
        nc, in_maps, core_ids=list(range(N_CORES)),
        trace=trace, trace_kwargs=trace_kwargs or {},
    )

    y = np.zeros((B * S, H), dtype=np.float32)
    for c in range(N_CORES):
        yT_c = res.results[c]["yT"]  # [H, CT] at scale SA*SW2
        for s, col0 in ((0, 0), (1, CH1)):
            e, lo, hi = assign[c][s]
            toks, wts = routes[e]
            toks, wts = toks[lo:hi], wts[lo:hi]
            y[toks] += (wts * np.float32(YSCALE))[:, None] * \
                yT_c[:, col0:col0 + len(toks)].T
    return y.reshape(B, S, H), res


def kernel(**inputs):
    y, _ = _run(inputs)
    return y
